# revision 36
# baseline (speedup 1.0000x reference)
"""LRU (Linear Recurrent Unit) single-step forward on 8 Trainium2 NeuronCores.

Math: with seq-len 1 the whole LRU step collapses algebraically to one GEMM:
    y[b,:] = W @ u[b] + bias
where
    W    = 2*C_re@diag(g)@B_re - 2*C_im@diag(g)@B_im + D          [DOUT, DIN]
    bias = 2*(C_re@(lam_re*x_re - lam_im*x_im)
              - C_im@(lam_re*x_im + lam_im*x_re))                  [DOUT]
    g = exp(gamma_log), lam = exp(-exp(nu_log)) * exp(i*exp(theta_log)).

The parameter fold (W, bias) is input-data independent, computed once on host
in float64.  The batch GEMM (99% of FLOPs and bytes) runs on the 8
NeuronCores, data-parallel over the batch: each core computes
y_shard^T = W @ u_shard^T (+bias).

Device kernel (per core), v2:
  - All GEMM operands ship as fp16 (u rounding ~2.4e-4 rel, W ~2.4e-4 —
    far inside the 2e-2 tolerance) and feed the PE directly (no upcast).
    fp16 matmul runs at the same 1 elem/cell/cycle rate as fp32r but
    halves SBUF + HBM traffic.
  - y returns as fp16 and is upcast on host: halves store traffic.
  - u for batch-tiles 1-3 arrives in three 1 MiB DMAs (vs 256 KiB tiles):
    better DMA efficiency, 4x fewer sem-waits stalling the PE queue.
  - Loads split across both HWDGE queues: W0+u0 blocks on the scalar queue
    (its engine preamble finishes ~1.3us before sync's), rest on sync;
    output stores on scalar.
  - PE warm-up junk matmuls gated only by two tiny DVE memsets, so they
    start at the top of the kernel body and the HAM clock gate (3.4us
    activity window) releases before the real MM stream begins.
  - PSUM->SBUF drains all on DVE (bias-add + fp16 cast); the scalar engine
    only issues store DMAs, so drains never queue behind DMA descriptor
    generation.
"""

import numpy as np

BATCH, DIN, DSTATE, DOUT = 16384, 1024, 2048, 1024
N_CORES = 8
B_SHARD = BATCH // N_CORES  # 2048 rows per core
P = 128                     # SBUF partitions
NB = 512                    # batch tile (moving free dim, max 512 per PSUM bank)
I_BLOCKS = DIN // P         # 8 contraction blocks
J_BLOCKS = DOUT // P        # 8 output-row blocks
B_TILES = B_SHARD // NB     # 4 batch tiles per core
N_WARM = 17                 # PE warm-up matmuls (HAM clock-gate release)

_CACHE = {}


def _build_nc():
    import concourse.mybir as mybir
    import concourse.tile as tile
    from concourse import bacc
    from concourse._compat import get_trn_type

    nc = bacc.Bacc(get_trn_type() or "TRN2", target_bir_lowering=False)
    f32 = mybir.dt.float32
    f16 = mybir.dt.float16

    # Combined (u batch-tile-0 block, W block) pairs: head[ib][p] holds the
    # 512 u0 values then the 1024 W^T columns for contraction block ib.  One
    # 384 KiB DMA per pair -> exactly one completion semaphore gates each
    # K-outer matmul group (avoids Tile's wait-merging pulling in later DMAs:
    # only 8 round-robin DMAHW lanes exist and matmuls carry 1 wait).
    head = nc.declare_dram_parameter("head", [I_BLOCKS, P, DOUT + NB], f16,
                                     isOutput=False)
    # u batch-tiles 1-3: one contiguous 1 MiB region per tile
    # (per partition: 8 KiB = all 8 ib chunks back to back).
    ubr = nc.declare_dram_parameter("ubr", [B_TILES - 1, P, I_BLOCKS * NB],
                                    f16, isOutput=False)
    bias = nc.declare_dram_parameter("bias", [P, J_BLOCKS], f32,
                                     isOutput=False)
    yb = nc.declare_dram_parameter("yb", [B_TILES, P, J_BLOCKS * NB], f16,
                                   isOutput=True)

    with tile.TileContext(nc) as tc:
        with (
            tc.tile_pool(name="consts", bufs=1) as consts,
            tc.tile_pool(name="upool", bufs=1) as upool,
            tc.tile_pool(name="ypool", bufs=1) as ypool,
            tc.tile_pool(name="psum", bufs=8, space="PSUM") as psum,
        ):
            # PE warm-up: the HAM clock gate keeps the PE at 1.2 GHz until it
            # has been busy a full ~3.4us activity window.  Junk matmuls gated
            # only on two tiny DVE memsets start right at kernel-body top and
            # release the gate before the real stream begins.
            warm_w = consts.tile([P, P], f16, tag="warm_w")
            warm_u = consts.tile([P, 256], f16, tag="warm_u")
            nc.vector.memset(warm_w[:], 0.0)
            nc.vector.memset(warm_u[:], 0.0)
            warm_p = psum.tile([P, NB], f32, tag="pt", name="warm_p")
            for _ in range(N_WARM):
                nc.tensor.matmul(warm_p[:, 0:256], warm_w[:], warm_u[:],
                                 start=True, stop=True)

            # Pair loads alternate across both HWDGE rings (sync first: its
            # first-byte latency is ~0.65us vs ~1.5us on scalar's first use);
            # K-outer group k becomes ready when ONE queue's FIFO reaches
            # pair k.  Emission order fixes the DMAHW lane round-robin:
            # pair0..pair7 land on the 8 distinct lanes; the later loads
            # (bias, ur*) wrap onto lanes whose first user is waited by an
            # instruction with no spare wait slots, so no harmful merges.
            # tiny bias load leads the sync ring: absorbs any first-use
            # wake-up latency before the critical pair0 transfer
            bias_t = consts.tile([P, J_BLOCKS], f32, tag="bias")
            nc.sync.dma_start(out=bias_t[:], in_=bias[:])
            pair_tiles = []
            for ib in range(I_BLOCKS):
                pt_ = consts.tile([P, DOUT + NB], f16, tag=f"pair{ib}",
                                  name=f"pair{ib}")
                q = nc.sync if ib % 2 == 0 else nc.scalar
                if ib == 0:
                    # split pair0: the first matmul group (jb0-3) gates on
                    # u0 + W columns 0:512 only (256 KiB, lands ~0.5us
                    # sooner); jb4-7 wait for the second half
                    h = NB + DOUT // 2
                    q.dma_start(out=pt_[:, 0:h], in_=head[0, :, 0:h])
                    q.dma_start(out=pt_[:, h:NB + DOUT],
                                in_=head[0, :, h:NB + DOUT])
                else:
                    q.dma_start(out=pt_[:], in_=head[ib])
                pair_tiles.append(pt_)
            ur_tiles = []
            for r in range(B_TILES - 1):
                ur = upool.tile([P, I_BLOCKS * NB], f16, tag=f"ur{r}",
                                name=f"ur{r}")
                q = nc.scalar if r == 0 else nc.sync
                q.dma_start(out=ur[:], in_=ubr[r])
                ur_tiles.append(ur)

            def w_block(ib, jb):
                return pair_tiles[ib][:, NB + jb * P:NB + (jb + 1) * P]

            def u0_block(ib):
                return pair_tiles[ib][:, 0:NB]

            y_tiles = [ypool.tile([P, J_BLOCKS * NB], f16, tag=f"y{bt}",
                                  name=f"y{bt}")
                       for bt in range(B_TILES)]

            act_id = mybir.ActivationFunctionType.Identity

            def drain_store(bt, jb, pt):
                """PSUM -> SBUF bias-add + fp16 cast on DVE, then store."""
                yt = y_tiles[bt]
                if bt == 1 and jb == 1:
                    # one early ACT drain: forces the activation-table load
                    # mid-stream (hidden), so the final ACT half-drain is fast
                    nc.scalar.activation(yt[:, jb * NB:(jb + 1) * NB], pt[:],
                                         act_id, bias=bias_t[:, jb:jb + 1])
                else:
                    nc.vector.tensor_scalar_add(yt[:, jb * NB:(jb + 1) * NB],
                                                pt[:], bias_t[:, jb:jb + 1])
                if bt == B_TILES - 1:
                    # last batch tile: store per jb (128 KiB), alternating
                    # queues so consecutive stores issue in parallel
                    q = nc.sync if jb % 2 == 0 else nc.scalar
                    q.dma_start(
                        out=yb[bt, :, jb * NB:(jb + 1) * NB],
                        in_=yt[:, jb * NB:(jb + 1) * NB])
                elif jb % 2 == 1:
                    nc.scalar.dma_start(
                        out=yb[bt, :, (jb - 1) * NB:(jb + 1) * NB],
                        in_=yt[:, (jb - 1) * NB:(jb + 1) * NB])

            # Batch tile 0 runs K-outer: all 8 PSUM groups in flight; each
            # arriving (W, u) block pair unlocks one matmul in every group.
            pts = [psum.tile([P, NB], f32, tag="pt", name=f"pt_0_{jb}")
                   for jb in range(J_BLOCKS)]
            for ib in range(I_BLOCKS):
                for jb in range(J_BLOCKS):
                    nc.tensor.matmul(
                        pts[jb][:],
                        w_block(ib, jb),
                        u0_block(ib),
                        start=(ib == 0),
                        stop=(ib == I_BLOCKS - 1),
                    )
            for jb in range(J_BLOCKS):
                drain_store(0, jb, pts[jb])

            # Batch tiles 1-3 run jb-outer so PSUM drains spread out.
            for bt in range(1, B_TILES):
                ur = ur_tiles[bt - 1]
                for jb in range(J_BLOCKS):
                    if bt == B_TILES - 1 and jb == J_BLOCKS - 1:
                        # Very last group: two half-width PSUM groups so the
                        # first half drains + stores while the second half's
                        # matmuls still run; the kernel tail after the last
                        # matmul is one ~380ns half-drain + one 32 KiB store.
                        h = NB // 2
                        yt = y_tiles[bt]
                        for half in range(2):
                            pth = psum.tile([P, NB], f32, tag="pt",
                                            name=f"pt_{bt}_{jb}_{half}")
                            for ib in range(I_BLOCKS):
                                nc.tensor.matmul(
                                    pth[:, 0:h],
                                    w_block(ib, jb),
                                    ur[:, ib * NB + half * h:
                                        ib * NB + half * h + h],
                                    start=(ib == 0),
                                    stop=(ib == I_BLOCKS - 1),
                                )
                            c0 = jb * NB + half * h
                            if half == 0:
                                nc.scalar.activation(
                                    yt[:, c0:c0 + h], pth[:, 0:h], act_id,
                                    bias=bias_t[:, jb:jb + 1])
                                nc.scalar.dma_start(
                                    out=yb[bt, :, c0:c0 + h],
                                    in_=yt[:, c0:c0 + h])
                            else:
                                nc.vector.tensor_scalar_add(
                                    yt[:, c0:c0 + h], pth[:, 0:h],
                                    bias_t[:, jb:jb + 1])
                                nc.sync.dma_start(
                                    out=yb[bt, :, c0:c0 + h],
                                    in_=yt[:, c0:c0 + h])
                        continue
                    pt = psum.tile([P, NB], f32, tag="pt",
                                   name=f"pt_{bt}_{jb}")
                    for ib in range(I_BLOCKS):
                        nc.tensor.matmul(
                            pt[:],
                            w_block(ib, jb),
                            ur[:, ib * NB:(ib + 1) * NB],
                            start=(ib == 0),
                            stop=(ib == I_BLOCKS - 1),
                        )
                    drain_store(bt, jb, pt)
    nc.compile()
    return nc


def _fold_params(x_re, x_im, nu_log, theta_log, gamma_log, B_re, B_im, C_re, C_im, D):
    """Fold the LRU parameters into (W^T [DIN, DOUT], bias [DOUT]) in float64."""
    nu = np.asarray(nu_log, np.float64)
    th = np.exp(np.asarray(theta_log, np.float64))
    lam_mod = np.exp(-np.exp(nu))
    lam_re = lam_mod * np.cos(th)
    lam_im = lam_mod * np.sin(th)
    g = np.exp(np.asarray(gamma_log, np.float64))
    C_re64 = np.asarray(C_re, np.float64)
    C_im64 = np.asarray(C_im, np.float64)
    W = (2.0 * ((C_re64 * g) @ np.asarray(B_re, np.float64))
         - 2.0 * ((C_im64 * g) @ np.asarray(B_im, np.float64))
         + np.asarray(D, np.float64))  # [DOUT, DIN]
    xr = np.asarray(x_re, np.float64)
    xi = np.asarray(x_im, np.float64)
    lx_re = lam_re * xr - lam_im * xi
    lx_im = lam_re * xi + lam_im * xr
    bias = 2.0 * (C_re64 @ lx_re - C_im64 @ lx_im)  # [DOUT]
    return W.T.astype(np.float32).copy(), bias.astype(np.float32)


def kernel(u_in, x_re, x_im, nu_log, theta_log, gamma_log, B_re, B_im,
           C_re, C_im, D, _trace=False):
    from concourse.bass_utils import run_bass_kernel_spmd

    wt_host, bias_host = _fold_params(
        x_re, x_im, nu_log, theta_log, gamma_log, B_re, B_im, C_re, C_im, D)
    wt16 = wt_host.astype(np.float16)
    bias2 = np.ascontiguousarray(bias_host.reshape(J_BLOCKS, P).T)  # [128, 8]

    u16 = np.asarray(u_in, np.float32).reshape(BATCH, DIN).astype(np.float16)
    core_ids = list(range(N_CORES))
    in_maps = []
    wt3 = wt16.reshape(I_BLOCKS, P, DOUT)
    for c in core_ids:
        shard = u16[c * B_SHARD:(c + 1) * B_SHARD]          # [2048, 1024]
        # head[ib, p] = [u0 block: shard[n, ib*P+p] | W^T row ib*P+p (1024)]
        ub0c = shard[:NB].reshape(NB, I_BLOCKS, P).transpose(1, 2, 0)
        headc = np.ascontiguousarray(
            np.concatenate([ub0c, wt3], axis=2))            # [8, 128, 1536]
        # ubr[r, p, ib*NB + n] = shard[(r+1)*NB + n, ib*P + p]
        ubrc = np.ascontiguousarray(
            shard[NB:].reshape(B_TILES - 1, NB, I_BLOCKS, P)
                 .transpose(0, 3, 2, 1)).reshape(B_TILES - 1, P,
                                                 I_BLOCKS * NB)
        in_maps.append({"head": headc, "ubr": ubrc, "bias": bias2})

    if "nc" not in _CACHE:
        _CACHE["nc"] = _build_nc()
    res = run_bass_kernel_spmd(_CACHE["nc"], in_maps, core_ids, trace=_trace)

    y = np.empty((BATCH, DOUT), np.float32)
    for c in core_ids:
        # yb[bt, p, jb*NB + n] = y_shard[bt*NB + n, jb*P + p]
        ybc = np.asarray(res.results[c]["yb"])
        y[c * B_SHARD:(c + 1) * B_SHARD] = (
            ybc.reshape(B_TILES, P, J_BLOCKS, NB).transpose(0, 3, 2, 1)
               .reshape(B_SHARD, DOUT).astype(np.float32))
    out = y.reshape(BATCH, 1, DOUT)
    if _trace:
        return out, res
    return out


# revision 38
# speedup vs baseline: 1.0109x; 1.0109x over previous
"""LRU (Linear Recurrent Unit) single-step forward on 8 Trainium2 NeuronCores.

Math: with seq-len 1 the whole LRU step collapses algebraically to one GEMM:
    y[b,:] = W @ u[b] + bias
where
    W    = 2*C_re@diag(g)@B_re - 2*C_im@diag(g)@B_im + D          [DOUT, DIN]
    bias = 2*(C_re@(lam_re*x_re - lam_im*x_im)
              - C_im@(lam_re*x_im + lam_im*x_re))                  [DOUT]
    g = exp(gamma_log), lam = exp(-exp(nu_log)) * exp(i*exp(theta_log)).

The parameter fold (W, bias) is input-data independent, computed once on host
in float64.  The batch GEMM (99% of FLOPs and bytes) runs on the 8
NeuronCores, data-parallel over the batch: each core computes
y_shard^T = W @ u_shard^T (+bias).

Device kernel (per core), v2:
  - All GEMM operands ship as fp16 (u rounding ~2.4e-4 rel, W ~2.4e-4 —
    far inside the 2e-2 tolerance) and feed the PE directly (no upcast).
    fp16 matmul runs at the same 1 elem/cell/cycle rate as fp32r but
    halves SBUF + HBM traffic.
  - y returns as fp16 and is upcast on host: halves store traffic.
  - u for batch-tiles 1-3 arrives in three 1 MiB DMAs (vs 256 KiB tiles):
    better DMA efficiency, 4x fewer sem-waits stalling the PE queue.
  - Loads split across both HWDGE queues: W0+u0 blocks on the scalar queue
    (its engine preamble finishes ~1.3us before sync's), rest on sync;
    output stores on scalar.
  - PE warm-up junk matmuls gated only by two tiny DVE memsets, so they
    start at the top of the kernel body and the HAM clock gate (3.4us
    activity window) releases before the real MM stream begins.
  - PSUM->SBUF drains all on DVE (bias-add + fp16 cast); the scalar engine
    only issues store DMAs, so drains never queue behind DMA descriptor
    generation.
"""

import numpy as np

BATCH, DIN, DSTATE, DOUT = 16384, 1024, 2048, 1024
N_CORES = 8
B_SHARD = BATCH // N_CORES  # 2048 rows per core
P = 128                     # SBUF partitions
NB = 512                    # batch tile (moving free dim, max 512 per PSUM bank)
I_BLOCKS = DIN // P         # 8 contraction blocks
J_BLOCKS = DOUT // P        # 8 output-row blocks
B_TILES = B_SHARD // NB     # 4 batch tiles per core
N_WARM = 17                 # PE warm-up matmuls (HAM clock-gate release)

_CACHE = {}


def _build_nc():
    import concourse.mybir as mybir
    import concourse.tile as tile
    from concourse import bacc
    from concourse._compat import get_trn_type

    nc = bacc.Bacc(get_trn_type() or "TRN2", target_bir_lowering=False)
    f32 = mybir.dt.float32
    f16 = mybir.dt.float16

    # Combined (u batch-tile-0 block, W block) pairs: head[ib][p] holds the
    # 512 u0 values then the 1024 W^T columns for contraction block ib.  One
    # 384 KiB DMA per pair -> exactly one completion semaphore gates each
    # K-outer matmul group (avoids Tile's wait-merging pulling in later DMAs:
    # only 8 round-robin DMAHW lanes exist and matmuls carry 1 wait).
    head = nc.declare_dram_parameter("head", [I_BLOCKS, P, DOUT + NB], f16,
                                     isOutput=False)
    # u batch-tiles 1-3: one contiguous 1 MiB region per tile
    # (per partition: 8 KiB = all 8 ib chunks back to back).
    ubr = nc.declare_dram_parameter("ubr", [B_TILES - 1, P, I_BLOCKS * NB],
                                    f16, isOutput=False)
    bias = nc.declare_dram_parameter("bias", [P, J_BLOCKS], f32,
                                     isOutput=False)
    yb = nc.declare_dram_parameter("yb", [B_TILES, P, J_BLOCKS * NB], f16,
                                   isOutput=True)

    with tile.TileContext(nc) as tc:
        with (
            tc.tile_pool(name="consts", bufs=1) as consts,
            tc.tile_pool(name="upool", bufs=1) as upool,
            tc.tile_pool(name="ypool", bufs=1) as ypool,
            tc.tile_pool(name="psum", bufs=8, space="PSUM") as psum,
        ):
            # PE warm-up: the HAM clock gate keeps the PE at 1.2 GHz until it
            # has been busy a full ~3.4us activity window.  Junk matmuls gated
            # only on two tiny DVE memsets start right at kernel-body top and
            # release the gate before the real stream begins.
            warm_w = consts.tile([P, P], f16, tag="warm_w")
            warm_u = consts.tile([P, 256], f16, tag="warm_u")
            nc.vector.memset(warm_w[:], 0.0)
            nc.vector.memset(warm_u[:], 0.0)
            warm_p = psum.tile([P, NB], f32, tag="pt", name="warm_p")
            for _ in range(N_WARM):
                nc.tensor.matmul(warm_p[:, 0:256], warm_w[:], warm_u[:],
                                 start=True, stop=True)

            # Pair loads alternate across both HWDGE rings (sync first: its
            # first-byte latency is ~0.65us vs ~1.5us on scalar's first use);
            # K-outer group k becomes ready when ONE queue's FIFO reaches
            # pair k.  Emission order fixes the DMAHW lane round-robin:
            # pair0..pair7 land on the 8 distinct lanes; the later loads
            # (bias, ur*) wrap onto lanes whose first user is waited by an
            # instruction with no spare wait slots, so no harmful merges.
            pair_tiles = []
            for ib in range(I_BLOCKS):
                pt_ = consts.tile([P, DOUT + NB], f16, tag=f"pair{ib}",
                                  name=f"pair{ib}")
                q = nc.sync if ib % 2 == 0 else nc.scalar
                if ib == 0:
                    # split pair0: the first matmul group (jb0-3) gates on
                    # u0 + W columns 0:512 only (256 KiB, lands ~0.5us
                    # sooner); jb4-7 wait for the second half
                    h = NB + DOUT // 2
                    q.dma_start(out=pt_[:, 0:h], in_=head[0, :, 0:h])
                    q.dma_start(out=pt_[:, h:NB + DOUT],
                                in_=head[0, :, h:NB + DOUT])
                else:
                    q.dma_start(out=pt_[:], in_=head[ib])
                pair_tiles.append(pt_)
            bias_t = consts.tile([P, J_BLOCKS], f32, tag="bias")
            nc.sync.dma_start(out=bias_t[:], in_=bias[:])
            ur_tiles = []
            for r in range(B_TILES - 1):
                ur = upool.tile([P, I_BLOCKS * NB], f16, tag=f"ur{r}",
                                name=f"ur{r}")
                q = nc.scalar if r == 0 else nc.sync
                q.dma_start(out=ur[:], in_=ubr[r])
                ur_tiles.append(ur)

            def w_block(ib, jb):
                return pair_tiles[ib][:, NB + jb * P:NB + (jb + 1) * P]

            def u0_block(ib):
                return pair_tiles[ib][:, 0:NB]

            y_tiles = [ypool.tile([P, J_BLOCKS * NB], f16, tag=f"y{bt}",
                                  name=f"y{bt}")
                       for bt in range(B_TILES)]

            act_id = mybir.ActivationFunctionType.Identity

            def drain_store(bt, jb, pt):
                """PSUM -> SBUF bias-add + fp16 cast on DVE, then store."""
                yt = y_tiles[bt]
                if bt == 1 and jb == 1:
                    # one early ACT drain: forces the activation-table load
                    # mid-stream (hidden), so the final ACT half-drain is fast
                    nc.scalar.activation(yt[:, jb * NB:(jb + 1) * NB], pt[:],
                                         act_id, bias=bias_t[:, jb:jb + 1])
                else:
                    nc.vector.tensor_scalar_add(yt[:, jb * NB:(jb + 1) * NB],
                                                pt[:], bias_t[:, jb:jb + 1])
                if bt == B_TILES - 1:
                    # last batch tile: store per jb (128 KiB), alternating
                    # queues so consecutive stores issue in parallel
                    q = nc.sync if jb % 2 == 0 else nc.scalar
                    q.dma_start(
                        out=yb[bt, :, jb * NB:(jb + 1) * NB],
                        in_=yt[:, jb * NB:(jb + 1) * NB])
                elif jb % 2 == 1:
                    nc.scalar.dma_start(
                        out=yb[bt, :, (jb - 1) * NB:(jb + 1) * NB],
                        in_=yt[:, (jb - 1) * NB:(jb + 1) * NB])

            # Batch tile 0 runs K-outer: all 8 PSUM groups in flight; each
            # arriving (W, u) block pair unlocks one matmul in every group.
            pts = [psum.tile([P, NB], f32, tag="pt", name=f"pt_0_{jb}")
                   for jb in range(J_BLOCKS)]
            for ib in range(I_BLOCKS):
                for jb in range(J_BLOCKS):
                    nc.tensor.matmul(
                        pts[jb][:],
                        w_block(ib, jb),
                        u0_block(ib),
                        start=(ib == 0),
                        stop=(ib == I_BLOCKS - 1),
                    )
            for jb in range(J_BLOCKS):
                drain_store(0, jb, pts[jb])

            # Batch tiles 1-3 run jb-outer so PSUM drains spread out.
            for bt in range(1, B_TILES):
                ur = ur_tiles[bt - 1]
                for jb in range(J_BLOCKS):
                    if bt == B_TILES - 1 and jb == J_BLOCKS - 1:
                        # Very last group: two half-width PSUM groups so the
                        # first half drains + stores while the second half's
                        # matmuls still run; the kernel tail after the last
                        # matmul is one ~380ns half-drain + one 32 KiB store.
                        h = NB // 2
                        yt = y_tiles[bt]
                        for half in range(2):
                            pth = psum.tile([P, NB], f32, tag="pt",
                                            name=f"pt_{bt}_{jb}_{half}")
                            for ib in range(I_BLOCKS):
                                nc.tensor.matmul(
                                    pth[:, 0:h],
                                    w_block(ib, jb),
                                    ur[:, ib * NB + half * h:
                                        ib * NB + half * h + h],
                                    start=(ib == 0),
                                    stop=(ib == I_BLOCKS - 1),
                                )
                            c0 = jb * NB + half * h
                            if half == 0:
                                nc.scalar.activation(
                                    yt[:, c0:c0 + h], pth[:, 0:h], act_id,
                                    bias=bias_t[:, jb:jb + 1])
                                nc.scalar.dma_start(
                                    out=yb[bt, :, c0:c0 + h],
                                    in_=yt[:, c0:c0 + h])
                            else:
                                nc.vector.tensor_scalar_add(
                                    yt[:, c0:c0 + h], pth[:, 0:h],
                                    bias_t[:, jb:jb + 1])
                                nc.sync.dma_start(
                                    out=yb[bt, :, c0:c0 + h],
                                    in_=yt[:, c0:c0 + h])
                        continue
                    pt = psum.tile([P, NB], f32, tag="pt",
                                   name=f"pt_{bt}_{jb}")
                    for ib in range(I_BLOCKS):
                        nc.tensor.matmul(
                            pt[:],
                            w_block(ib, jb),
                            ur[:, ib * NB:(ib + 1) * NB],
                            start=(ib == 0),
                            stop=(ib == I_BLOCKS - 1),
                        )
                    drain_store(bt, jb, pt)
    nc.compile()
    return nc


def _fold_params(x_re, x_im, nu_log, theta_log, gamma_log, B_re, B_im, C_re, C_im, D):
    """Fold the LRU parameters into (W^T [DIN, DOUT], bias [DOUT]) in float64."""
    nu = np.asarray(nu_log, np.float64)
    th = np.exp(np.asarray(theta_log, np.float64))
    lam_mod = np.exp(-np.exp(nu))
    lam_re = lam_mod * np.cos(th)
    lam_im = lam_mod * np.sin(th)
    g = np.exp(np.asarray(gamma_log, np.float64))
    C_re64 = np.asarray(C_re, np.float64)
    C_im64 = np.asarray(C_im, np.float64)
    W = (2.0 * ((C_re64 * g) @ np.asarray(B_re, np.float64))
         - 2.0 * ((C_im64 * g) @ np.asarray(B_im, np.float64))
         + np.asarray(D, np.float64))  # [DOUT, DIN]
    xr = np.asarray(x_re, np.float64)
    xi = np.asarray(x_im, np.float64)
    lx_re = lam_re * xr - lam_im * xi
    lx_im = lam_re * xi + lam_im * xr
    bias = 2.0 * (C_re64 @ lx_re - C_im64 @ lx_im)  # [DOUT]
    return W.T.astype(np.float32).copy(), bias.astype(np.float32)


def kernel(u_in, x_re, x_im, nu_log, theta_log, gamma_log, B_re, B_im,
           C_re, C_im, D, _trace=False):
    from concourse.bass_utils import run_bass_kernel_spmd

    wt_host, bias_host = _fold_params(
        x_re, x_im, nu_log, theta_log, gamma_log, B_re, B_im, C_re, C_im, D)
    wt16 = wt_host.astype(np.float16)
    bias2 = np.ascontiguousarray(bias_host.reshape(J_BLOCKS, P).T)  # [128, 8]

    u16 = np.asarray(u_in, np.float32).reshape(BATCH, DIN).astype(np.float16)
    core_ids = list(range(N_CORES))
    in_maps = []
    wt3 = wt16.reshape(I_BLOCKS, P, DOUT)
    for c in core_ids:
        shard = u16[c * B_SHARD:(c + 1) * B_SHARD]          # [2048, 1024]
        # head[ib, p] = [u0 block: shard[n, ib*P+p] | W^T row ib*P+p (1024)]
        ub0c = shard[:NB].reshape(NB, I_BLOCKS, P).transpose(1, 2, 0)
        headc = np.ascontiguousarray(
            np.concatenate([ub0c, wt3], axis=2))            # [8, 128, 1536]
        # ubr[r, p, ib*NB + n] = shard[(r+1)*NB + n, ib*P + p]
        ubrc = np.ascontiguousarray(
            shard[NB:].reshape(B_TILES - 1, NB, I_BLOCKS, P)
                 .transpose(0, 3, 2, 1)).reshape(B_TILES - 1, P,
                                                 I_BLOCKS * NB)
        in_maps.append({"head": headc, "ubr": ubrc, "bias": bias2})

    if "nc" not in _CACHE:
        _CACHE["nc"] = _build_nc()
    res = run_bass_kernel_spmd(_CACHE["nc"], in_maps, core_ids, trace=_trace)

    y = np.empty((BATCH, DOUT), np.float32)
    for c in core_ids:
        # yb[bt, p, jb*NB + n] = y_shard[bt*NB + n, jb*P + p]
        ybc = np.asarray(res.results[c]["yb"])
        y[c * B_SHARD:(c + 1) * B_SHARD] = (
            ybc.reshape(B_TILES, P, J_BLOCKS, NB).transpose(0, 3, 2, 1)
               .reshape(B_SHARD, DOUT).astype(np.float32))
    out = y.reshape(BATCH, 1, DOUT)
    if _trace:
        return out, res
    return out
